# revision 26
# baseline (speedup 1.0000x reference)
"""CoordAtt Trainium2 Bass kernel.

Reference computation (per batch n, c=256, h=w=64, mip=8):
    xs   = x + residual                      (bilinear resize of residual at
                                              identical shape is the identity)
    y    = concat(mean_w(xs), mean_h(xs))    -> [c, h+w]
    y    = hswish(BN(w1 @ y + b1))           -> [mip, h+w]
    a_h  = sigmoid(w2 @ y[:, :h] + b2)       -> [c, h]
    a_w  = sigmoid(w3 @ y[:, h:] + b3)       -> [c, w]
    out  = 2*xs*a_h*a_w + 2*residual*(1 - a_h*a_w)
         = 2*(a_h*a_w*x + residual)          (algebraically identical)

Kernel strategy (8 cores, data-parallel over batch n: 2 batches/core):
  * conv-before-pool: pooling and the 1x1 conv are both linear, so compute
    y_conv = w1^T @ x + w1^T @ res on the TensorEngine (PSUM accumulation
    gives the x+res add for free), then pool the tiny (mip, h*w) result.
    Matmuls run in float32r mode: fp32 data at full PE rate.
  * BN folds into one per-partition scale/bias activation op.
  * final elementwise tail is only 3 ops/element, split across VectorE (DVE)
    and GpSimd on h-half tiles (separate SBUF tensors per engine --
    concurrent DVE+GpSimd in-place writes to one tensor hang the device),
    all in-place in the input tiles.
  * two emission phases (all pool/attention work for every batch first, then
    all finals) so batch i+1's pipeline overlaps batch i's elementwise tail.
"""

import numpy as np

import concourse.bacc as bacc
import concourse.mybir as mybir
from concourse.tile import TileContext
from concourse.bass_utils import run_bass_kernel_spmd

F32 = mybir.dt.float32
F32R = mybir.dt.float32r
BF16 = mybir.dt.bfloat16
Alu = mybir.AluOpType
Act = mybir.ActivationFunctionType
AX = mybir.AxisListType

N_CORES = 8
N, C, H, W = 16, 256, 64, 64
NLOC = N // N_CORES           # batches per core
MIP = 8
EPS = 1e-5
HW = H * W                    # 4096 free columns per (batch, c-chunk)
NCHUNK = C // 128             # c-chunk count (2)
NHALF = 2                     # h-half split of each chunk tile
HCOL = HW // NHALF            # 2048 columns per half tile
SEG = 2                       # conv psum segments per batch: 2 x 2048 cols
SEGH = H // SEG               # h rows per segment (32)
SEGCOL = SEGH * W             # columns per segment (2048)

# final elementwise: which (chunk, half) units go to GpSimd (rest on DVE),
# per batch: give GpSimd the late halves of batch 0 but the early halves of
# batch 1, so the tail of the last batch lands on the faster DVE
GP_UNITS_BY_BATCH = (frozenset({(0, 0), (1, 0)}), frozenset({(0, 0), (1, 0)}))
FINAL_BF16 = False            # bf16 finals lose the DVE 2x mode to the
                              # stride-0 broadcast operand -- not worth it

ALL_STAGES = frozenset({"conv", "pools", "mlp", "final_dve", "final_gp"})


def build_module(stages=ALL_STAGES):
    nc = bacc.Bacc("TRN2", target_bir_lowering=False)

    x_d = nc.dram_tensor("x", (NLOC, C, H, W), F32, kind="ExternalInput")
    r_d = nc.dram_tensor("residual", (NLOC, C, H, W), F32, kind="ExternalInput")
    w1_d = nc.dram_tensor("w1", (MIP, C), F32, kind="ExternalInput")
    b1_d = nc.dram_tensor("b1", (MIP,), F32, kind="ExternalInput")
    gamma_d = nc.dram_tensor("bn_gamma", (MIP,), F32, kind="ExternalInput")
    beta_d = nc.dram_tensor("bn_beta", (MIP,), F32, kind="ExternalInput")
    mean_d = nc.dram_tensor("bn_mean", (MIP,), F32, kind="ExternalInput")
    var_d = nc.dram_tensor("bn_var", (MIP,), F32, kind="ExternalInput")
    w2_d = nc.dram_tensor("w2", (C, MIP), F32, kind="ExternalInput")
    b2_d = nc.dram_tensor("b2", (C,), F32, kind="ExternalInput")
    w3_d = nc.dram_tensor("w3", (C, MIP), F32, kind="ExternalInput")
    b3_d = nc.dram_tensor("b3", (C,), F32, kind="ExternalInput")
    out_d = nc.dram_tensor("out", (NLOC, C, H, W), F32, kind="ExternalOutput")

    with TileContext(nc) as tc:
        with (
            tc.tile_pool(name="big", bufs=1) as big,
            tc.tile_pool(name="small", bufs=1) as small,
            tc.tile_pool(name="work", bufs=2) as work,
            tc.tile_pool(name="psum_y", bufs=1, space="PSUM") as psum_y_pool,
            tc.tile_pool(name="psum_a", bufs=2, space="PSUM") as psum_a_pool,
        ):
            # ---- replicated constants ----
            # w1 chunk-transposed: (c128, mip) per c-chunk
            w1t = []
            for k in range(NCHUNK):
                tf = small.tile([128, MIP], F32, name=f"w1tf{k}", tag=f"w1tf{k}")
                nc.scalar.dma_start(tf[:], w1_d[:, k * 128:(k + 1) * 128].rearrange("m c -> c m"))
                t = small.tile([128, MIP], BF16, name=f"w1t{k}", tag=f"w1t{k}")
                nc.scalar.copy(t[:], tf[:])
                w1t.append(t)
            # w2/w3 transposed: (mip, C)
            w2t = small.tile([MIP, C], F32, tag="w2t")
            nc.scalar.dma_start(w2t[:], w2_d.rearrange("o m -> m o"))
            w3t = small.tile([MIP, C], F32, tag="w3t")
            nc.scalar.dma_start(w3t[:], w3_d.rearrange("o m -> m o"))
            # b2/b3 per-partition: (128, chunk)
            b2t = small.tile([128, NCHUNK], F32, tag="b2t")
            nc.scalar.dma_start(b2t[:], b2_d.rearrange("(k p) -> p k", p=128))
            b3t = small.tile([128, NCHUNK], F32, tag="b3t")
            nc.scalar.dma_start(b3t[:], b3_d.rearrange("(k p) -> p k", p=128))
            # BN constants, (mip, 1) per-partition scalars
            bn_in = small.tile([MIP, 5], F32, tag="bn_in")
            for i, d in enumerate((var_d, gamma_d, beta_d, mean_d, b1_d)):
                nc.scalar.dma_start(bn_in[:, i:i + 1], d[:].unsqueeze(1))
            var_c = bn_in[:, 0:1]
            gamma_c = bn_in[:, 1:2]
            beta_c = bn_in[:, 2:3]
            mean_c = bn_in[:, 3:4]
            b1_c = bn_in[:, 4:5]

            consts = small.tile([128, 2], F32, tag="consts")
            nc.vector.memset(consts[:, 0:1], EPS)
            nc.vector.memset(consts[:, 1:2], 3.0)

            bn_t = small.tile([MIP, 4], F32, tag="bn_t")
            sv = bn_t[:, 0:1]       # sqrt(var+eps)
            inv = bn_t[:, 1:2]      # gamma / sqrt(var+eps)
            scale_p = bn_t[:, 2:3]  # inv / W   (pool-sum -> mean fold)
            bias_p = bn_t[:, 3:4]   # (b1 - mean) * inv + beta
            nc.scalar.activation(sv, var_c, Act.Sqrt, bias=consts[:MIP, 0:1], scale=1.0)
            nc.vector.reciprocal(inv, sv)
            nc.vector.tensor_tensor(inv, inv, gamma_c, Alu.mult)
            nc.vector.tensor_scalar_mul(scale_p, inv, 1.0 / W)
            nc.vector.tensor_tensor(bias_p, b1_c, mean_c, Alu.subtract)
            nc.vector.scalar_tensor_tensor(bias_p, bias_p, inv, beta_c, Alu.mult, Alu.add)

            xt = {}
            rt = {}
            ah2 = {}
            aw = {}
            awb_bf = {}
            xbf_all = {}

            # ---- phase 1 (per batch): load, conv, pools, attention ----
            for b in range(NLOC):
                xbf = {}
                rbf = {}
                xbf_all[b] = xbf
                for j in range(NHALF):
                    js = slice(j * HCOL, (j + 1) * HCOL)
                    for k in range(NCHUNK):
                        cs = slice(k * 128, (k + 1) * 128)
                        t = big.tile([128, HCOL], F32, name=f"x_{b}_{k}_{j}", tag=f"x{b}{k}{j}")
                        nc.sync.dma_start(t[:], x_d[b, cs].rearrange("c h w -> c (h w)")[:, js])
                        xt[b, k, j] = t
                        t = big.tile([128, HCOL], F32, name=f"r_{b}_{k}_{j}", tag=f"r{b}{k}{j}")
                        nc.sync.dma_start(t[:], r_d[b, cs].rearrange("c h w -> c (h w)")[:, js])
                        rt[b, k, j] = t
                        with tc.high_priority():
                            tb = big.tile([128, HCOL], BF16, name=f"xb_{b}_{k}_{j}", tag="xb", bufs=8)
                            nc.scalar.copy(tb[:], xt[b, k, j][:])
                            xbf[k, j] = tb
                            tb = big.tile([128, HCOL], BF16, name=f"rb_{b}_{k}_{j}", tag="rb", bufs=4)
                            nc.scalar.copy(tb[:], rt[b, k, j][:])
                            rbf[k, j] = tb

                # conv (c -> mip) + implicit x+res via PSUM accumulation,
                # then directional pool sums.  The a_h attention path only
                # needs row pools of its own h-segment, so it is computed per
                # segment and the a_h multiply of the finals starts before the
                # whole batch has even arrived.
                assert "conv" in stages and "pools" in stages and "mlp" in stages
                yh_sum = work.tile([MIP, H], F32, name=f"yh_{b}", tag="yh")
                ywp = work.tile([MIP, SEG * W], F32, name=f"ywp_{b}", tag="ywp")
                aht_k = []
                for k in range(NCHUNK):
                    aht = work.tile([128, H], F32, name=f"ah_{b}_{k}", tag=f"ah{k}")
                    aht_k.append(aht)
                    ah2[b, k] = aht
                for s in range(SEG):
                    # segment s covers h rows [s*SEGH, (s+1)*SEGH) = half tile s
                    hs = slice(s * SEGH, (s + 1) * SEGH)
                    with tc.high_priority():
                        ypsum = psum_y_pool.tile([MIP, SEGCOL], F32, name=f"yp_{b}_{s}", tag="yp")
                        for jj in range(0, SEGCOL, 512):
                            srcs = [(k, t) for k in range(NCHUNK)
                                    for t in (xbf[k, s], rbf[k, s])]
                            for i, (k, src) in enumerate(srcs):
                                nc.tensor.matmul(
                                    ypsum[:, jj:jj + 512],
                                    w1t[k][:, :MIP],
                                    src[:, jj:jj + 512],
                                    start=(i == 0),
                                    stop=(i == len(srcs) - 1),
                                )
                        # partial column sums first: they gate a_w, which
                        # is the critical path of the whole batch
                        nc.vector.reduce_sum(
                            ywp[:, s * W:(s + 1) * W],
                            ypsum.rearrange("m (h w) -> m w h", h=SEGH),
                            axis=AX.X,
                        )
                        # row sums (over w) for this segment's h rows
                        nc.vector.reduce_sum(
                            yh_sum[:, hs],
                            ypsum.rearrange("m (h w) -> m h w", h=SEGH),
                            axis=AX.X,
                        )
                        # staggered a_h path for this segment's rows:
                        # BN + hswish + 1x1 conv + sigmoid (x2 folded in)
                        ybn_s = work.tile([MIP, SEGH], F32, name=f"ybnh_{b}_{s}", tag="ybnh", bufs=4)
                        u_s = work.tile([MIP, SEGH], F32, name=f"uh_{b}_{s}", tag="uh", bufs=4)
                        v_s = work.tile([MIP, SEGH], F32, name=f"vh_{b}_{s}", tag="vh", bufs=4)
                        nc.scalar.activation(ybn_s[:], yh_sum[:, hs], Act.Identity, bias=bias_p, scale=scale_p)
                        nc.scalar.activation(u_s[:], ybn_s[:], Act.Relu, bias=consts[:MIP, 1:2], scale=1.0)
                        nc.vector.tensor_scalar_min(u_s[:], u_s[:], 6.0)
                        nc.vector.scalar_tensor_tensor(v_s[:], u_s[:], 1.0 / 6.0, ybn_s[:], Alu.mult, Alu.mult)
                        for k in range(NCHUNK):
                            cs = slice(k * 128, (k + 1) * 128)
                            ahp = psum_a_pool.tile([128, SEGH], F32, name=f"ahp_{b}_{s}_{k}", tag="ahp")
                            nc.tensor.matmul(ahp[:], w2t[:, cs], v_s[:], start=True, stop=True)
                            nc.scalar.activation(aht_k[k][:, hs], ahp[:], Act.Sigmoid, bias=b2t[:, k:k + 1], scale=1.0)
                            nc.scalar.mul(aht_k[k][:, hs], aht_k[k][:, hs], 2.0)
                    # staggered first final multiply on half tile s:
                    # t = x * a_h2   (in bf16 in-place in the cast tile, or
                    # fp32 in-place in the x tile)
                    for k in range(NCHUNK):
                        if f"final_{'gp' if (k, s) in GP_UNITS_BY_BATCH[b] else 'dve'}" not in stages:
                            continue
                        if FINAL_BF16:
                            ahb_t = work.tile([128, H], BF16, name=f"ahb_{b}_{k}", tag=f"ahb{k}")
                            nc.scalar.copy(ahb_t[:, hs], aht_k[k][:, hs])
                            xs_ = xbf[k, s].rearrange("p (h w) -> p h w", h=SEGH)
                            ahb = ahb_t[:, hs].unsqueeze(2).broadcast_to((128, SEGH, W))
                        else:
                            xs_ = xt[b, k, s].rearrange("p (h w) -> p h w", h=SEGH)
                            ahb = aht_k[k][:, hs].unsqueeze(2).broadcast_to((128, SEGH, W))
                        if (k, s) in GP_UNITS_BY_BATCH[b]:
                            nc.gpsimd.tensor_tensor(xs_, xs_, ahb, Alu.mult)
                        else:
                            nc.vector.tensor_tensor(xs_, xs_, ahb, Alu.mult)

                # a_w path needs column pools over all h: finish it now
                with tc.high_priority():
                    yw_sum = work.tile([MIP, W], F32, name=f"yw_{b}", tag="yw")
                    nc.vector.tensor_tensor(yw_sum[:], ywp[:, 0:W], ywp[:, W:2 * W], Alu.add)
                    ybn_w = work.tile([MIP, W], F32, name=f"ybnw_{b}", tag="ybnw")
                    u_w = work.tile([MIP, W], F32, name=f"uw_{b}", tag="uw")
                    v_w = work.tile([MIP, W], F32, name=f"vw_{b}", tag="vw")
                    nc.scalar.activation(ybn_w[:], yw_sum[:], Act.Identity, bias=bias_p, scale=scale_p)
                    nc.scalar.activation(u_w[:], ybn_w[:], Act.Relu, bias=consts[:MIP, 1:2], scale=1.0)
                    nc.vector.tensor_scalar_min(u_w[:], u_w[:], 6.0)
                    nc.vector.scalar_tensor_tensor(v_w[:], u_w[:], 1.0 / 6.0, ybn_w[:], Alu.mult, Alu.mult)
                    for k in range(NCHUNK):
                        cs = slice(k * 128, (k + 1) * 128)
                        awt = work.tile([128, W], F32, name=f"aw_{b}_{k}", tag=f"aw{k}")
                        awp = psum_a_pool.tile([128, W], F32, name=f"awp_{b}_{k}", tag="awp")
                        nc.tensor.matmul(awp[:], w3t[:, cs], v_w[:], start=True, stop=True)
                        nc.scalar.activation(awt[:], awp[:], Act.Sigmoid, bias=b3t[:, k:k + 1], scale=1.0)
                        aw[b, k] = awt
                        if FINAL_BF16:
                            awb_t = work.tile([128, W], BF16, name=f"awb_{b}_{k}", tag=f"awb{k}")
                            nc.scalar.copy(awb_t[:], awt[:])
                            awb_bf[b, k] = awb_t

            # ---- phase 2 (per batch): final elementwise + store ----
            # out = (2*a_h*a_w)*x + 2*res
            for b in range(NLOC):
                for k in range(NCHUNK):
                    cs = slice(k * 128, (k + 1) * 128)
                    od = out_d[b, cs].rearrange("c h w -> c (h w)")
                    for j in range(NHALF):
                        h0 = j * (H // NHALF)
                        h1 = (j + 1) * (H // NHALF)
                        nh = h1 - h0
                        eng = "gp" if (k, j) in GP_UNITS_BY_BATCH[b] else "dve"
                        if f"final_{eng}" not in stages:
                            continue
                        rs_ = rt[b, k, j].rearrange("p (h w) -> p h w", h=nh)
                        if FINAL_BF16:
                            xs_ = xbf_all[b][k, j].rearrange("p (h w) -> p h w", h=nh)
                            awb_t = awb_bf[b, k]
                            awb = awb_t[:].unsqueeze(1).broadcast_to((128, nh, W))
                        else:
                            xs_ = xt[b, k, j].rearrange("p (h w) -> p h w", h=nh)
                            awb = aw[b, k].unsqueeze(1).broadcast_to((128, nh, W))
                        if eng == "dve":
                            nc.vector.tensor_tensor(xs_, xs_, awb, Alu.mult)
                            nc.vector.scalar_tensor_tensor(rs_, rs_, 2.0, xs_, Alu.mult, Alu.add)
                        else:
                            nc.gpsimd.tensor_tensor(xs_, xs_, awb, Alu.mult)
                            nc.gpsimd.tensor_scalar_mul(rs_, rs_, 2.0)
                            nc.gpsimd.tensor_tensor(rs_, rs_, xs_, Alu.add)

                        # result lives in rt[b, k, j]
                        nc.sync.dma_start(od[:, j * HCOL:(j + 1) * HCOL], rt[b, k, j][:])

    nc.compile()
    return nc


_NC_CACHE = None


def _get_module():
    global _NC_CACHE
    if _NC_CACHE is None:
        _NC_CACHE = build_module()
    return _NC_CACHE


def make_in_maps(inputs):
    reps = {k: np.ascontiguousarray(v) for k, v in inputs.items()
            if k not in ("x", "residual")}
    in_maps = []
    for core in range(N_CORES):
        bs = slice(core * NLOC, (core + 1) * NLOC)
        m = {"x": np.ascontiguousarray(inputs["x"][bs]),
             "residual": np.ascontiguousarray(inputs["residual"][bs])}
        m.update(reps)
        in_maps.append(m)
    return in_maps


def run_spmd(nc, in_maps):
    res = run_bass_kernel_spmd(nc, in_maps, core_ids=list(range(N_CORES)))
    return np.concatenate([res.results[c]["out"] for c in range(N_CORES)], axis=0)


def kernel(**inputs):
    inputs = {k: np.asarray(v) for k, v in inputs.items()}
    nc = _get_module()
    return run_spmd(nc, make_in_maps(inputs))


# revision 34
# speedup vs baseline: 1.1182x; 1.1182x over previous
"""CoordAtt Trainium2 Bass kernel.

Reference computation (per batch n, c=256, h=w=64, mip=8):
    xs   = x + residual                      (bilinear resize of residual at
                                              identical shape is the identity)
    y    = concat(mean_w(xs), mean_h(xs))    -> [c, h+w]
    y    = hswish(BN(w1 @ y + b1))           -> [mip, h+w]
    a_h  = sigmoid(w2 @ y[:, :h] + b2)       -> [c, h]
    a_w  = sigmoid(w3 @ y[:, h:] + b3)       -> [c, w]
    out  = 2*xs*a_h*a_w + 2*residual*(1 - a_h*a_w)
         = 2*(a_h*a_w*x + residual)          (algebraically identical)

Kernel strategy (8 cores, data-parallel over batch n: 2 batches/core):
  * conv-before-pool: pooling and the 1x1 conv are both linear, so compute
    y_conv = w1^T @ x + w1^T @ res on the TensorEngine (PSUM accumulation
    gives the x+res add for free), then pool the tiny (mip, h*w) result.
    Matmuls run in float32r mode: fp32 data at full PE rate.
  * BN folds into one per-partition scale/bias activation op.
  * final elementwise tail is only 3 ops/element, split across VectorE (DVE)
    and GpSimd on h-half tiles (separate SBUF tensors per engine --
    concurrent DVE+GpSimd in-place writes to one tensor hang the device),
    all in-place in the input tiles.
  * two emission phases (all pool/attention work for every batch first, then
    all finals) so batch i+1's pipeline overlaps batch i's elementwise tail.
"""

import numpy as np

import concourse.bacc as bacc
import concourse.mybir as mybir
from concourse.tile import TileContext
from concourse.bass_utils import run_bass_kernel_spmd

F32 = mybir.dt.float32
F32R = mybir.dt.float32r
BF16 = mybir.dt.bfloat16
Alu = mybir.AluOpType
Act = mybir.ActivationFunctionType
AX = mybir.AxisListType

N_CORES = 8
N, C, H, W = 16, 256, 64, 64
NLOC = N // N_CORES           # batches per core
MIP = 8
EPS = 1e-5
HW = H * W                    # 4096 free columns per (batch, c-chunk)
NCHUNK = C // 128             # c-chunk count (2)
NHALF = 2                     # h-half split of each chunk tile
HCOL = HW // NHALF            # 2048 columns per half tile
SEG = 4                       # conv psum segments per batch: 4 x 1024 cols
SEGH = H // SEG               # h rows per segment (16)
SEGCOL = SEGH * W             # columns per segment (1024)
HALFH = H // NHALF            # h rows per half tile (32)

# final elementwise: which (chunk, half) units go to GpSimd (rest on DVE),
# per batch: give GpSimd the late halves of batch 0 but the early halves of
# batch 1, so the tail of the last batch lands on the faster DVE
GP_UNITS_BY_BATCH = (frozenset({(0, 0), (1, 1)}), frozenset({(0, 0)}))
FINAL_BF16 = False            # bf16 finals lose the DVE 2x mode to the
                              # stride-0 broadcast operand -- not worth it

ALL_STAGES = frozenset({"conv", "pools", "mlp", "final_dve", "final_gp"})


def build_module(stages=ALL_STAGES):
    nc = bacc.Bacc("TRN2", target_bir_lowering=False)

    x_d = nc.dram_tensor("x", (NLOC, C, H, W), F32, kind="ExternalInput")
    r_d = nc.dram_tensor("residual", (NLOC, C, H, W), F32, kind="ExternalInput")
    w1_d = nc.dram_tensor("w1", (MIP, C), F32, kind="ExternalInput")
    b1_d = nc.dram_tensor("b1", (MIP,), F32, kind="ExternalInput")
    gamma_d = nc.dram_tensor("bn_gamma", (MIP,), F32, kind="ExternalInput")
    beta_d = nc.dram_tensor("bn_beta", (MIP,), F32, kind="ExternalInput")
    mean_d = nc.dram_tensor("bn_mean", (MIP,), F32, kind="ExternalInput")
    var_d = nc.dram_tensor("bn_var", (MIP,), F32, kind="ExternalInput")
    w2_d = nc.dram_tensor("w2", (C, MIP), F32, kind="ExternalInput")
    b2_d = nc.dram_tensor("b2", (C,), F32, kind="ExternalInput")
    w3_d = nc.dram_tensor("w3", (C, MIP), F32, kind="ExternalInput")
    b3_d = nc.dram_tensor("b3", (C,), F32, kind="ExternalInput")
    out_d = nc.dram_tensor("out", (NLOC, C, H, W), F32, kind="ExternalOutput")

    with TileContext(nc) as tc:
        with (
            tc.tile_pool(name="big", bufs=1) as big,
            tc.tile_pool(name="small", bufs=1) as small,
            tc.tile_pool(name="work", bufs=2) as work,
            tc.tile_pool(name="psum_y", bufs=2, space="PSUM") as psum_y_pool,
            tc.tile_pool(name="psum_a", bufs=2, space="PSUM") as psum_a_pool,
        ):
            # ---- replicated constants ----
            # w1 chunk-transposed: (c128, mip) per c-chunk
            w1t = []
            for k in range(NCHUNK):
                tf = small.tile([128, MIP], F32, name=f"w1tf{k}", tag=f"w1tf{k}")
                nc.scalar.dma_start(tf[:], w1_d[:, k * 128:(k + 1) * 128].rearrange("m c -> c m"))
                t = small.tile([128, MIP], BF16, name=f"w1t{k}", tag=f"w1t{k}")
                nc.scalar.copy(t[:], tf[:])
                w1t.append(t)
            # w2/w3 transposed: (mip, C)
            w2t = small.tile([MIP, C], F32, tag="w2t")
            nc.scalar.dma_start(w2t[:], w2_d.rearrange("o m -> m o"))
            w3t = small.tile([MIP, C], F32, tag="w3t")
            nc.scalar.dma_start(w3t[:], w3_d.rearrange("o m -> m o"))
            # b2/b3 per-partition: (128, chunk)
            b2t = small.tile([128, NCHUNK], F32, tag="b2t")
            nc.scalar.dma_start(b2t[:], b2_d.rearrange("(k p) -> p k", p=128))
            b3t = small.tile([128, NCHUNK], F32, tag="b3t")
            nc.scalar.dma_start(b3t[:], b3_d.rearrange("(k p) -> p k", p=128))
            # BN constants, (mip, 1) per-partition scalars
            bn_in = small.tile([MIP, 5], F32, tag="bn_in")
            for i, d in enumerate((var_d, gamma_d, beta_d, mean_d, b1_d)):
                nc.scalar.dma_start(bn_in[:, i:i + 1], d[:].unsqueeze(1))
            var_c = bn_in[:, 0:1]
            gamma_c = bn_in[:, 1:2]
            beta_c = bn_in[:, 2:3]
            mean_c = bn_in[:, 3:4]
            b1_c = bn_in[:, 4:5]

            consts = small.tile([128, 2], F32, tag="consts")
            nc.vector.memset(consts[:, 0:1], EPS)
            nc.vector.memset(consts[:, 1:2], 3.0)

            bn_t = small.tile([MIP, 4], F32, tag="bn_t")
            sv = bn_t[:, 0:1]       # sqrt(var+eps)
            inv = bn_t[:, 1:2]      # gamma / sqrt(var+eps)
            scale_p = bn_t[:, 2:3]  # inv / W   (pool-sum -> mean fold)
            bias_p = bn_t[:, 3:4]   # (b1 - mean) * inv + beta
            nc.scalar.activation(sv, var_c, Act.Sqrt, bias=consts[:MIP, 0:1], scale=1.0)
            nc.vector.reciprocal(inv, sv)
            nc.vector.tensor_tensor(inv, inv, gamma_c, Alu.mult)
            nc.vector.tensor_scalar_mul(scale_p, inv, 1.0 / W)
            nc.vector.tensor_tensor(bias_p, b1_c, mean_c, Alu.subtract)
            nc.vector.scalar_tensor_tensor(bias_p, bias_p, inv, beta_c, Alu.mult, Alu.add)

            xt = {}
            rt = {}
            ah2 = {}
            aw = {}
            awb_bf = {}
            xbf_all = {}

            # ---- phase 1 (per batch): load, conv, pools, attention ----
            for b in range(NLOC):
                xbf = {}
                rbf = {}
                xbf_all[b] = xbf
                for j in range(NHALF):
                    js = slice(j * HCOL, (j + 1) * HCOL)
                    for k in range(NCHUNK):
                        cs = slice(k * 128, (k + 1) * 128)
                        t = big.tile([128, HCOL], F32, name=f"x_{b}_{k}_{j}", tag=f"x{b}{k}{j}")
                        nc.sync.dma_start(t[:], x_d[b, cs].rearrange("c h w -> c (h w)")[:, js])
                        xt[b, k, j] = t
                        t = big.tile([128, HCOL], F32, name=f"r_{b}_{k}_{j}", tag=f"r{b}{k}{j}")
                        nc.sync.dma_start(t[:], r_d[b, cs].rearrange("c h w -> c (h w)")[:, js])
                        rt[b, k, j] = t
                        tb = big.tile([128, HCOL], BF16, name=f"xb_{b}_{k}_{j}", tag="xb", bufs=8)
                        nc.scalar.copy(tb[:], xt[b, k, j][:])
                        xbf[k, j] = tb
                        tb = big.tile([128, HCOL], BF16, name=f"rb_{b}_{k}_{j}", tag="rb", bufs=4)
                        nc.scalar.copy(tb[:], rt[b, k, j][:])
                        rbf[k, j] = tb

                # conv (c -> mip) + implicit x+res via PSUM accumulation,
                # then directional pool sums.  The a_h attention path only
                # needs row pools of its own h-segment, so it is computed per
                # segment and the a_h multiply of the finals starts before the
                # whole batch has even arrived.
                assert "conv" in stages and "pools" in stages and "mlp" in stages
                yh_sum = work.tile([MIP, H], F32, name=f"yh_{b}", tag="yh")
                ywp = work.tile([MIP, SEG * W], F32, name=f"ywp_{b}", tag="ywp")
                aht_k = []
                for k in range(NCHUNK):
                    aht = work.tile([128, H], F32, name=f"ah_{b}_{k}", tag=f"ah{k}")
                    aht_k.append(aht)
                    ah2[b, k] = aht
                for s in range(SEG):
                    # psum segment s covers h rows [s*SEGH, (s+1)*SEGH);
                    # half tile jh = s // 2 (two psum segments per half tile)
                    jh = s // (SEG // NHALF)
                    soff = (s % (SEG // NHALF)) * SEGCOL
                    ypsum = psum_y_pool.tile([MIP, SEGCOL], F32, name=f"yp_{b}_{s}", tag="yp")
                    for jj in range(0, SEGCOL, 512):
                        srcs = [(k, t) for k in range(NCHUNK)
                                for t in (xbf[k, jh], rbf[k, jh])]
                        for i, (k, src) in enumerate(srcs):
                            nc.tensor.matmul(
                                ypsum[:, jj:jj + 512],
                                w1t[k][:, :MIP],
                                src[:, soff + jj:soff + jj + 512],
                                start=(i == 0),
                                stop=(i == len(srcs) - 1),
                            )
                    # partial column sums first: they gate a_w, which is the
                    # critical path of the whole batch
                    nc.vector.reduce_sum(
                        ywp[:, s * W:(s + 1) * W],
                        ypsum.rearrange("m (h w) -> m w h", h=SEGH),
                        axis=AX.X,
                    )
                    # row sums (over w) for this segment's h rows
                    nc.vector.reduce_sum(
                        yh_sum[:, s * SEGH:(s + 1) * SEGH],
                        ypsum.rearrange("m (h w) -> m h w", h=SEGH),
                        axis=AX.X,
                    )
                for s in range(NHALF):
                    # staggered a_h path for this half's rows:
                    # BN + hswish + 1x1 conv + sigmoid (x2 folded in)
                    hs = slice(s * HALFH, (s + 1) * HALFH)
                    ybn_s = work.tile([MIP, HALFH], F32, name=f"ybnh_{b}_{s}", tag="ybnh", bufs=4)
                    u_s = work.tile([MIP, HALFH], F32, name=f"uh_{b}_{s}", tag="uh", bufs=4)
                    v_s = work.tile([MIP, HALFH], F32, name=f"vh_{b}_{s}", tag="vh", bufs=4)
                    nc.scalar.activation(ybn_s[:], yh_sum[:, hs], Act.Identity, bias=bias_p, scale=scale_p)
                    nc.scalar.activation(u_s[:], ybn_s[:], Act.Relu, bias=consts[:MIP, 1:2], scale=1.0)
                    nc.vector.tensor_scalar_min(u_s[:], u_s[:], 6.0)
                    nc.vector.scalar_tensor_tensor(v_s[:], u_s[:], 1.0 / 6.0, ybn_s[:], Alu.mult, Alu.mult)
                    for k in range(NCHUNK):
                        cs = slice(k * 128, (k + 1) * 128)
                        ahp = psum_a_pool.tile([128, HALFH], F32, name=f"ahp_{b}_{s}_{k}", tag="ahp")
                        nc.tensor.matmul(ahp[:], w2t[:, cs], v_s[:], start=True, stop=True)
                        nc.scalar.activation(aht_k[k][:, hs], ahp[:], Act.Sigmoid, bias=b2t[:, k:k + 1], scale=1.0)
                        nc.scalar.mul(aht_k[k][:, hs], aht_k[k][:, hs], 2.0)
                    # staggered first final multiply on half tile s:
                    # t = x * a_h2   (in bf16 in-place in the cast tile, or
                    # fp32 in-place in the x tile)
                    for k in range(NCHUNK):
                        if f"final_{'gp' if (k, s) in GP_UNITS_BY_BATCH[b] else 'dve'}" not in stages:
                            continue
                        xs_ = xt[b, k, s].rearrange("p (h w) -> p h w", h=HALFH)
                        ahb = aht_k[k][:, hs].unsqueeze(2).broadcast_to((128, HALFH, W))
                        if (k, s) in GP_UNITS_BY_BATCH[b]:
                            nc.gpsimd.tensor_tensor(xs_, xs_, ahb, Alu.mult)
                        else:
                            nc.vector.tensor_tensor(xs_, xs_, ahb, Alu.mult)

                # a_w path needs column pools over all h: finish it now
                if True:
                    yw_sum = work.tile([MIP, W], F32, name=f"yw_{b}", tag="yw")
                    nc.vector.tensor_tensor(ywp[:, 0:W], ywp[:, 0:W], ywp[:, W:2 * W], Alu.add)
                    nc.vector.tensor_tensor(ywp[:, 2 * W:3 * W], ywp[:, 2 * W:3 * W], ywp[:, 3 * W:4 * W], Alu.add)
                    nc.vector.tensor_tensor(yw_sum[:], ywp[:, 0:W], ywp[:, 2 * W:3 * W], Alu.add)
                    ybn_w = work.tile([MIP, W], F32, name=f"ybnw_{b}", tag="ybnw")
                    u_w = work.tile([MIP, W], F32, name=f"uw_{b}", tag="uw")
                    v_w = work.tile([MIP, W], F32, name=f"vw_{b}", tag="vw")
                    nc.scalar.activation(ybn_w[:], yw_sum[:], Act.Identity, bias=bias_p, scale=scale_p)
                    nc.scalar.activation(u_w[:], ybn_w[:], Act.Relu, bias=consts[:MIP, 1:2], scale=1.0)
                    nc.vector.tensor_scalar_min(u_w[:], u_w[:], 6.0)
                    nc.vector.scalar_tensor_tensor(v_w[:], u_w[:], 1.0 / 6.0, ybn_w[:], Alu.mult, Alu.mult)
                    for k in range(NCHUNK):
                        cs = slice(k * 128, (k + 1) * 128)
                        awt = work.tile([128, W], F32, name=f"aw_{b}_{k}", tag=f"aw{k}")
                        awp = psum_a_pool.tile([128, W], F32, name=f"awp_{b}_{k}", tag="awp")
                        nc.tensor.matmul(awp[:], w3t[:, cs], v_w[:], start=True, stop=True)
                        nc.scalar.activation(awt[:], awp[:], Act.Sigmoid, bias=b3t[:, k:k + 1], scale=1.0)
                        aw[b, k] = awt
                        if FINAL_BF16:
                            awb_t = work.tile([128, W], BF16, name=f"awb_{b}_{k}", tag=f"awb{k}")
                            nc.scalar.copy(awb_t[:], awt[:])
                            awb_bf[b, k] = awb_t

            # ---- phase 2 (per batch): final elementwise + store ----
            # out = (2*a_h*a_w)*x + 2*res
            for b in range(NLOC):
                for k in range(NCHUNK):
                    cs = slice(k * 128, (k + 1) * 128)
                    od = out_d[b, cs].rearrange("c h w -> c (h w)")
                    for j in range(NHALF):
                        h0 = j * (H // NHALF)
                        h1 = (j + 1) * (H // NHALF)
                        nh = h1 - h0
                        eng = "gp" if (k, j) in GP_UNITS_BY_BATCH[b] else "dve"
                        if f"final_{eng}" not in stages:
                            continue
                        rs_ = rt[b, k, j].rearrange("p (h w) -> p h w", h=nh)
                        if FINAL_BF16:
                            xs_ = xbf_all[b][k, j].rearrange("p (h w) -> p h w", h=nh)
                            awb_t = awb_bf[b, k]
                            awb = awb_t[:].unsqueeze(1).broadcast_to((128, nh, W))
                        else:
                            xs_ = xt[b, k, j].rearrange("p (h w) -> p h w", h=nh)
                            awb = aw[b, k].unsqueeze(1).broadcast_to((128, nh, W))
                        if eng == "dve":
                            nc.vector.tensor_tensor(xs_, xs_, awb, Alu.mult)
                            nc.vector.scalar_tensor_tensor(rs_, rs_, 2.0, xs_, Alu.mult, Alu.add)
                        else:
                            nc.gpsimd.tensor_tensor(xs_, xs_, awb, Alu.mult)
                            nc.gpsimd.tensor_scalar_mul(rs_, rs_, 2.0)
                            nc.gpsimd.tensor_tensor(rs_, rs_, xs_, Alu.add)

                        # result lives in rt[b, k, j]
                        nc.sync.dma_start(od[:, j * HCOL:(j + 1) * HCOL], rt[b, k, j][:])

    nc.compile()
    return nc


_NC_CACHE = None


def _get_module():
    global _NC_CACHE
    if _NC_CACHE is None:
        _NC_CACHE = build_module()
    return _NC_CACHE


def make_in_maps(inputs):
    reps = {k: np.ascontiguousarray(v) for k, v in inputs.items()
            if k not in ("x", "residual")}
    in_maps = []
    for core in range(N_CORES):
        bs = slice(core * NLOC, (core + 1) * NLOC)
        m = {"x": np.ascontiguousarray(inputs["x"][bs]),
             "residual": np.ascontiguousarray(inputs["residual"][bs])}
        m.update(reps)
        in_maps.append(m)
    return in_maps


def run_spmd(nc, in_maps):
    res = run_bass_kernel_spmd(nc, in_maps, core_ids=list(range(N_CORES)))
    return np.concatenate([res.results[c]["out"] for c in range(N_CORES)], axis=0)


def kernel(**inputs):
    inputs = {k: np.asarray(v) for k, v in inputs.items()}
    nc = _get_module()
    return run_spmd(nc, make_in_maps(inputs))


# revision 38
# speedup vs baseline: 1.1340x; 1.0141x over previous
"""CoordAtt Trainium2 Bass kernel.

Reference computation (per batch n, c=256, h=w=64, mip=8):
    xs   = x + residual                      (bilinear resize of residual at
                                              identical shape is the identity)
    y    = concat(mean_w(xs), mean_h(xs))    -> [c, h+w]
    y    = hswish(BN(w1 @ y + b1))           -> [mip, h+w]
    a_h  = sigmoid(w2 @ y[:, :h] + b2)       -> [c, h]
    a_w  = sigmoid(w3 @ y[:, h:] + b3)       -> [c, w]
    out  = 2*xs*a_h*a_w + 2*residual*(1 - a_h*a_w)
         = 2*(a_h*a_w*x + residual)          (algebraically identical)

Kernel strategy (8 cores, data-parallel over batch n: 2 batches/core):
  * conv-before-pool: pooling and the 1x1 conv are both linear, so compute
    y_conv = w1^T @ x + w1^T @ res on the TensorEngine (PSUM accumulation
    gives the x+res add for free), then pool the tiny (mip, h*w) result.
    Matmuls run in float32r mode: fp32 data at full PE rate.
  * BN folds into one per-partition scale/bias activation op.
  * final elementwise tail is only 3 ops/element, split across VectorE (DVE)
    and GpSimd on h-half tiles (separate SBUF tensors per engine --
    concurrent DVE+GpSimd in-place writes to one tensor hang the device),
    all in-place in the input tiles.
  * two emission phases (all pool/attention work for every batch first, then
    all finals) so batch i+1's pipeline overlaps batch i's elementwise tail.
"""

import numpy as np

import concourse.bacc as bacc
import concourse.mybir as mybir
from concourse.tile import TileContext
from concourse.bass_utils import run_bass_kernel_spmd

F32 = mybir.dt.float32
F32R = mybir.dt.float32r
BF16 = mybir.dt.bfloat16
Alu = mybir.AluOpType
Act = mybir.ActivationFunctionType
AX = mybir.AxisListType

N_CORES = 8
N, C, H, W = 16, 256, 64, 64
NLOC = N // N_CORES           # batches per core
MIP = 8
EPS = 1e-5
HW = H * W                    # 4096 free columns per (batch, c-chunk)
NCHUNK = C // 128             # c-chunk count (2)
NHALF = 2                     # h-half split of each chunk tile
HCOL = HW // NHALF            # 2048 columns per half tile
SEG = 4                       # conv psum segments per batch: 4 x 1024 cols
SEGH = H // SEG               # h rows per segment (16)
SEGCOL = SEGH * W             # columns per segment (1024)
HALFH = H // NHALF            # h rows per half tile (32)

# final elementwise: which (chunk, half) units go to GpSimd (rest on DVE),
# per batch: give GpSimd the late halves of batch 0 but the early halves of
# batch 1, so the tail of the last batch lands on the faster DVE
GP_UNITS_BY_BATCH = (frozenset({(0, 0), (1, 1)}), frozenset({(0, 0)}))
FINAL_BF16 = False            # bf16 finals lose the DVE 2x mode to the
                              # stride-0 broadcast operand -- not worth it

ALL_STAGES = frozenset({"conv", "pools", "mlp", "final_dve", "final_gp"})


def build_module(stages=ALL_STAGES):
    nc = bacc.Bacc("TRN2", target_bir_lowering=False)

    x_d = nc.dram_tensor("x", (NLOC, C, H, W), F32, kind="ExternalInput")
    r_d = nc.dram_tensor("residual", (NLOC, C, H, W), F32, kind="ExternalInput")
    w1_d = nc.dram_tensor("w1", (MIP, C), F32, kind="ExternalInput")
    b1_d = nc.dram_tensor("b1", (MIP,), F32, kind="ExternalInput")
    gamma_d = nc.dram_tensor("bn_gamma", (MIP,), F32, kind="ExternalInput")
    beta_d = nc.dram_tensor("bn_beta", (MIP,), F32, kind="ExternalInput")
    mean_d = nc.dram_tensor("bn_mean", (MIP,), F32, kind="ExternalInput")
    var_d = nc.dram_tensor("bn_var", (MIP,), F32, kind="ExternalInput")
    w2_d = nc.dram_tensor("w2", (C, MIP), F32, kind="ExternalInput")
    b2_d = nc.dram_tensor("b2", (C,), F32, kind="ExternalInput")
    w3_d = nc.dram_tensor("w3", (C, MIP), F32, kind="ExternalInput")
    b3_d = nc.dram_tensor("b3", (C,), F32, kind="ExternalInput")
    out_d = nc.dram_tensor("out", (NLOC, C, H, W), F32, kind="ExternalOutput")

    with TileContext(nc) as tc:
        with (
            tc.tile_pool(name="big", bufs=1) as big,
            tc.tile_pool(name="small", bufs=1) as small,
            tc.tile_pool(name="work", bufs=2) as work,
            tc.tile_pool(name="psum_y", bufs=3, space="PSUM") as psum_y_pool,
            tc.tile_pool(name="psum_a", bufs=1, space="PSUM") as psum_a_pool,
        ):
            # ---- replicated constants ----
            # w1 chunk-transposed: (c128, mip) per c-chunk
            w1t = []
            for k in range(NCHUNK):
                tf = small.tile([128, MIP], F32, name=f"w1tf{k}", tag=f"w1tf{k}")
                nc.scalar.dma_start(tf[:], w1_d[:, k * 128:(k + 1) * 128].rearrange("m c -> c m"))
                t = small.tile([128, MIP], BF16, name=f"w1t{k}", tag=f"w1t{k}")
                nc.scalar.copy(t[:], tf[:])
                w1t.append(t)
            # w2/w3 transposed: (mip, C)
            w2t = small.tile([MIP, C], F32, tag="w2t")
            nc.scalar.dma_start(w2t[:], w2_d.rearrange("o m -> m o"))
            w3t = small.tile([MIP, C], F32, tag="w3t")
            nc.scalar.dma_start(w3t[:], w3_d.rearrange("o m -> m o"))
            # b2/b3 per-partition: (128, chunk)
            b2t = small.tile([128, NCHUNK], F32, tag="b2t")
            nc.scalar.dma_start(b2t[:], b2_d.rearrange("(k p) -> p k", p=128))
            b3t = small.tile([128, NCHUNK], F32, tag="b3t")
            nc.scalar.dma_start(b3t[:], b3_d.rearrange("(k p) -> p k", p=128))
            # BN constants, (mip, 1) per-partition scalars
            bn_in = small.tile([MIP, 5], F32, tag="bn_in")
            for i, d in enumerate((var_d, gamma_d, beta_d, mean_d, b1_d)):
                nc.scalar.dma_start(bn_in[:, i:i + 1], d[:].unsqueeze(1))
            var_c = bn_in[:, 0:1]
            gamma_c = bn_in[:, 1:2]
            beta_c = bn_in[:, 2:3]
            mean_c = bn_in[:, 3:4]
            b1_c = bn_in[:, 4:5]

            consts = small.tile([128, 2], F32, tag="consts")
            nc.vector.memset(consts[:, 0:1], EPS)
            nc.vector.memset(consts[:, 1:2], 3.0)

            bn_t = small.tile([MIP, 4], F32, tag="bn_t")
            sv = bn_t[:, 0:1]       # sqrt(var+eps)
            inv = bn_t[:, 1:2]      # gamma / sqrt(var+eps)
            scale_p = bn_t[:, 2:3]  # inv / W   (pool-sum -> mean fold)
            bias_p = bn_t[:, 3:4]   # (b1 - mean) * inv + beta
            nc.scalar.activation(sv, var_c, Act.Sqrt, bias=consts[:MIP, 0:1], scale=1.0)
            nc.vector.reciprocal(inv, sv)
            nc.vector.tensor_tensor(inv, inv, gamma_c, Alu.mult)
            nc.vector.tensor_scalar_mul(scale_p, inv, 1.0 / W)
            nc.vector.tensor_tensor(bias_p, b1_c, mean_c, Alu.subtract)
            nc.vector.scalar_tensor_tensor(bias_p, bias_p, inv, beta_c, Alu.mult, Alu.add)

            xt = {}
            rt = {}
            ah2 = {}
            aw = {}
            awb_bf = {}
            xbf_all = {}

            # ---- phase 1 (per batch): load, conv, pools, attention ----
            for b in range(NLOC):
                xbf = {}
                rbf = {}
                xbf_all[b] = xbf
                for j in range(NHALF):
                    js = slice(j * HCOL, (j + 1) * HCOL)
                    for k in range(NCHUNK):
                        cs = slice(k * 128, (k + 1) * 128)
                        t = big.tile([128, HCOL], F32, name=f"x_{b}_{k}_{j}", tag=f"x{b}{k}{j}")
                        nc.sync.dma_start(t[:], x_d[b, cs].rearrange("c h w -> c (h w)")[:, js])
                        xt[b, k, j] = t
                        t = big.tile([128, HCOL], F32, name=f"r_{b}_{k}_{j}", tag=f"r{b}{k}{j}")
                        nc.sync.dma_start(t[:], r_d[b, cs].rearrange("c h w -> c (h w)")[:, js])
                        rt[b, k, j] = t
                        tb = big.tile([128, HCOL], BF16, name=f"xb_{b}_{k}_{j}", tag="xb", bufs=8)
                        nc.scalar.copy(tb[:], xt[b, k, j][:])
                        xbf[k, j] = tb
                        tb = big.tile([128, HCOL], BF16, name=f"rb_{b}_{k}_{j}", tag="rb", bufs=4)
                        nc.scalar.copy(tb[:], rt[b, k, j][:])
                        rbf[k, j] = tb

                # conv (c -> mip) + implicit x+res via PSUM accumulation,
                # then directional pool sums.  The a_h attention path only
                # needs row pools of its own h-segment, so it is computed per
                # segment and the a_h multiply of the finals starts before the
                # whole batch has even arrived.
                assert "conv" in stages and "pools" in stages and "mlp" in stages
                yh_sum = work.tile([MIP, H], F32, name=f"yh_{b}", tag="yh")
                ywp = work.tile([MIP, SEG * W], F32, name=f"ywp_{b}", tag="ywp")
                aht_k = []
                for k in range(NCHUNK):
                    aht = work.tile([128, H], F32, name=f"ah_{b}_{k}", tag=f"ah{k}")
                    aht_k.append(aht)
                    ah2[b, k] = aht
                for s in range(SEG):
                    # psum segment s covers h rows [s*SEGH, (s+1)*SEGH);
                    # half tile jh = s // 2 (two psum segments per half tile)
                    jh = s // (SEG // NHALF)
                    soff = (s % (SEG // NHALF)) * SEGCOL
                    ypsum = psum_y_pool.tile([MIP, SEGCOL], F32, name=f"yp_{b}_{s}", tag="yp")
                    for jj in range(0, SEGCOL, 512):
                        srcs = [(k, t) for k in range(NCHUNK)
                                for t in (xbf[k, jh], rbf[k, jh])]
                        for i, (k, src) in enumerate(srcs):
                            nc.tensor.matmul(
                                ypsum[:, jj:jj + 512],
                                w1t[k][:, :MIP],
                                src[:, soff + jj:soff + jj + 512],
                                start=(i == 0),
                                stop=(i == len(srcs) - 1),
                            )
                    # partial column sums first: they gate a_w, which is the
                    # critical path of the whole batch
                    nc.vector.reduce_sum(
                        ywp[:, s * W:(s + 1) * W],
                        ypsum.rearrange("m (h w) -> m w h", h=SEGH),
                        axis=AX.X,
                    )
                    # row sums (over w) for this segment's h rows
                    nc.vector.reduce_sum(
                        yh_sum[:, s * SEGH:(s + 1) * SEGH],
                        ypsum.rearrange("m (h w) -> m h w", h=SEGH),
                        axis=AX.X,
                    )
                for s in range(NHALF):
                    # staggered a_h path for this half's rows:
                    # BN + hswish + 1x1 conv + sigmoid (x2 folded in)
                    hs = slice(s * HALFH, (s + 1) * HALFH)
                    ybn_s = work.tile([MIP, HALFH], F32, name=f"ybnh_{b}_{s}", tag="ybnh", bufs=4)
                    u_s = work.tile([MIP, HALFH], F32, name=f"uh_{b}_{s}", tag="uh", bufs=4)
                    v_s = work.tile([MIP, HALFH], F32, name=f"vh_{b}_{s}", tag="vh", bufs=4)
                    nc.scalar.activation(ybn_s[:], yh_sum[:, hs], Act.Identity, bias=bias_p, scale=scale_p)
                    nc.scalar.activation(u_s[:], ybn_s[:], Act.Relu, bias=consts[:MIP, 1:2], scale=1.0)
                    nc.vector.tensor_scalar_min(u_s[:], u_s[:], 6.0)
                    nc.vector.scalar_tensor_tensor(v_s[:], u_s[:], 1.0 / 6.0, ybn_s[:], Alu.mult, Alu.mult)
                    for k in range(NCHUNK):
                        cs = slice(k * 128, (k + 1) * 128)
                        ahp = psum_a_pool.tile([128, HALFH], F32, name=f"ahp_{b}_{s}_{k}", tag="ahp")
                        nc.tensor.matmul(ahp[:], w2t[:, cs], v_s[:], start=True, stop=True)
                        nc.scalar.activation(aht_k[k][:, hs], ahp[:], Act.Sigmoid, bias=b2t[:, k:k + 1], scale=1.0)
                        nc.scalar.mul(aht_k[k][:, hs], aht_k[k][:, hs], 2.0)
                    # staggered first final multiply on half tile s:
                    # t = x * a_h2   (in bf16 in-place in the cast tile, or
                    # fp32 in-place in the x tile)
                    for k in range(NCHUNK):
                        if f"final_{'gp' if (k, s) in GP_UNITS_BY_BATCH[b] else 'dve'}" not in stages:
                            continue
                        xs_ = xt[b, k, s].rearrange("p (h w) -> p h w", h=HALFH)
                        ahb = aht_k[k][:, hs].unsqueeze(2).broadcast_to((128, HALFH, W))
                        if (k, s) in GP_UNITS_BY_BATCH[b]:
                            nc.gpsimd.tensor_tensor(xs_, xs_, ahb, Alu.mult)
                        else:
                            nc.vector.tensor_tensor(xs_, xs_, ahb, Alu.mult)

                # a_w path needs column pools over all h: finish it now
                if True:
                    yw_sum = work.tile([MIP, W], F32, name=f"yw_{b}", tag="yw")
                    nc.vector.tensor_tensor(ywp[:, 0:W], ywp[:, 0:W], ywp[:, W:2 * W], Alu.add)
                    nc.vector.tensor_tensor(ywp[:, 2 * W:3 * W], ywp[:, 2 * W:3 * W], ywp[:, 3 * W:4 * W], Alu.add)
                    nc.vector.tensor_tensor(yw_sum[:], ywp[:, 0:W], ywp[:, 2 * W:3 * W], Alu.add)
                    ybn_w = work.tile([MIP, W], F32, name=f"ybnw_{b}", tag="ybnw")
                    u_w = work.tile([MIP, W], F32, name=f"uw_{b}", tag="uw")
                    v_w = work.tile([MIP, W], F32, name=f"vw_{b}", tag="vw")
                    nc.scalar.activation(ybn_w[:], yw_sum[:], Act.Identity, bias=bias_p, scale=scale_p)
                    nc.scalar.activation(u_w[:], ybn_w[:], Act.Relu, bias=consts[:MIP, 1:2], scale=1.0)
                    nc.vector.tensor_scalar_min(u_w[:], u_w[:], 6.0)
                    nc.vector.scalar_tensor_tensor(v_w[:], u_w[:], 1.0 / 6.0, ybn_w[:], Alu.mult, Alu.mult)
                    for k in range(NCHUNK):
                        cs = slice(k * 128, (k + 1) * 128)
                        awt = work.tile([128, W], F32, name=f"aw_{b}_{k}", tag=f"aw{k}")
                        awp = psum_a_pool.tile([128, W], F32, name=f"awp_{b}_{k}", tag="awp")
                        nc.tensor.matmul(awp[:], w3t[:, cs], v_w[:], start=True, stop=True)
                        nc.scalar.activation(awt[:], awp[:], Act.Sigmoid, bias=b3t[:, k:k + 1], scale=1.0)
                        aw[b, k] = awt
                        if FINAL_BF16:
                            awb_t = work.tile([128, W], BF16, name=f"awb_{b}_{k}", tag=f"awb{k}")
                            nc.scalar.copy(awb_t[:], awt[:])
                            awb_bf[b, k] = awb_t

            # ---- phase 2 (per batch): final elementwise + store ----
            # out = (2*a_h*a_w)*x + 2*res
            for b in range(NLOC):
                for k in range(NCHUNK):
                    cs = slice(k * 128, (k + 1) * 128)
                    od = out_d[b, cs].rearrange("c h w -> c (h w)")
                    for j in range(NHALF):
                        h0 = j * (H // NHALF)
                        h1 = (j + 1) * (H // NHALF)
                        nh = h1 - h0
                        eng = "gp" if (k, j) in GP_UNITS_BY_BATCH[b] else "dve"
                        if f"final_{eng}" not in stages:
                            continue
                        # last batch: quarter-granularity so the output
                        # DMAs start before the whole half tile is done
                        nq = 2 if b == NLOC - 1 else 1
                        qh = nh // nq
                        for q in range(nq):
                            qs = slice(q * qh, (q + 1) * qh)
                            xs_ = xt[b, k, j].rearrange("p (h w) -> p h w", h=nh)[:, qs, :]
                            rs_ = rt[b, k, j].rearrange("p (h w) -> p h w", h=nh)[:, qs, :]
                            awb = aw[b, k].unsqueeze(1).broadcast_to((128, qh, W))
                            if eng == "dve":
                                nc.vector.tensor_tensor(xs_, xs_, awb, Alu.mult)
                                nc.vector.scalar_tensor_tensor(rs_, rs_, 2.0, xs_, Alu.mult, Alu.add)
                            else:
                                nc.gpsimd.tensor_tensor(xs_, xs_, awb, Alu.mult)
                                nc.gpsimd.tensor_scalar_mul(rs_, rs_, 2.0)
                                nc.gpsimd.tensor_tensor(rs_, rs_, xs_, Alu.add)
                            nc.sync.dma_start(
                                od[:, j * HCOL + q * qh * W: j * HCOL + (q + 1) * qh * W],
                                rt[b, k, j][:, q * qh * W:(q + 1) * qh * W])

    nc.compile()
    return nc


_NC_CACHE = None


def _get_module():
    global _NC_CACHE
    if _NC_CACHE is None:
        _NC_CACHE = build_module()
    return _NC_CACHE


def make_in_maps(inputs):
    reps = {k: np.ascontiguousarray(v) for k, v in inputs.items()
            if k not in ("x", "residual")}
    in_maps = []
    for core in range(N_CORES):
        bs = slice(core * NLOC, (core + 1) * NLOC)
        m = {"x": np.ascontiguousarray(inputs["x"][bs]),
             "residual": np.ascontiguousarray(inputs["residual"][bs])}
        m.update(reps)
        in_maps.append(m)
    return in_maps


def run_spmd(nc, in_maps):
    res = run_bass_kernel_spmd(nc, in_maps, core_ids=list(range(N_CORES)))
    return np.concatenate([res.results[c]["out"] for c in range(N_CORES)], axis=0)


def kernel(**inputs):
    inputs = {k: np.asarray(v) for k, v in inputs.items()}
    nc = _get_module()
    return run_spmd(nc, make_in_maps(inputs))
